# revision 12
# baseline (speedup 1.0000x reference)
"""DispersionLoss Trainium2 kernel.

Computes mean over i<j pairs of exp(-||z_i - z_j||) for z [8192, 512] fp32,
distributed over 8 NeuronCores.

Strategy (identical SPMD program on all 8 cores; per-core behavior comes only
from input data):
  - Host rotates z by c*1024 rows for core c, so each core's 1024 rows sit at
    rotated positions 0..1023 and its circulant band of columns is the static
    range 0..5119. Each unordered pair {a,b} with forward separation
    s in [1, 4095] is computed exactly once somewhere in the fleet; the 4096
    pairs at separation exactly 4096 are added on the host in fp64.
  - Per core: 8 i-tiles x 9 j-units of [128, 512]. TensorE computes
    s = dot(z_i, z_j) - (sq_i + sq_j)/2 via 4 K=128 matmuls from a resident
    transposed band plus one K=1 augmented matmul (ones x -sq_j/2).
  - ScalarE (single ACT table set natural_log_exp_and_others, no reloads):
      pass1: L = Ln(-2*s + (sq_i + eps))   [= ln(d2)]
      pass2: d = Exp(0.5*L)                [= sqrt(d2)]
      pass3: e = Exp(-d), accum_out -> per-partition sums
  - GpSimd affine_select masks the band edges (j <= i and j - i >= 4096) by
    overwriting L with +170 so exp(-exp(85)) underflows to exactly 0.
  - Host: fp64 sum of the 8 cores' [128, 8] partial sums, undo the eps bias,
    add the separation-4096 pairs, divide by n(n-1)/2.
"""

import sys

sys.path.insert(0, "/opt/trn_rl_repo")

import numpy as np

N = 8192
D = 512
NCORES = 8
ROWS = N // NCORES          # 1024 rows per core
TI = ROWS // 128            # 8 i-tiles per core
NU = 9                      # j-units of 512 per i-tile
BANDC = (NU + 1) * 512      # 5120 resident band columns
EPS = 0.01                  # diagonal-safety shift added to d2 via the bias
MASK_FILL = 170.0           # L fill for masked elements: exp(-exp(85)) == 0
TAU = 1.0

_CACHE = {}


def make_split_drain_tc():
    """TileContext subclass whose kernel-tail drain splits its sem waits into
    individual 1-wait NOPs (walrus rejects many waits on one instruction)."""
    from concourse import tile, mybir
    from concourse.vector_clock import ScopedClock

    class SplitDrainTileContext(tile.TileContext):
        def _drain_and_barrier(self, tick_clock, wait_clock):
            drain_inst = self.nc.sync.drain()
            wait_clock.add_sem_waits(
                drain_inst.ins, ScopedClock({None: tick_clock.global_clock})
            )
            si = drain_inst.ins.sync_info
            if si is not None and len(si.on_wait) > 1:
                waits = list(si.on_wait)
                drain_inst.ins.sync_info = mybir.SyncInfo(
                    on_wait=[], on_update=list(si.on_update)
                )
                for w in waits:
                    nop = self.nc.sync.nop(nofuse=True)
                    nop.ins.sync_info = mybir.SyncInfo(on_wait=[w], on_update=[])
            self.nc.all_engine_barrier()
            assert self.sems is not None
            popped = self.nc._tile_sem_poison_stack.pop()
            assert popped is self._sem_poison
            self.nc.clear_and_free_semaphores(list(self.sems.allocated().values()))
            self.nc.all_engine_barrier()

    return SplitDrainTileContext


def strip_pe_self_waits(nc):
    """Drop same-engine self-waits from compute instructions that carry more
    than one wait. Each engine executes and retires its own instruction stream
    in order (PE matmuls are pc-monotone; ACT/DVE are strict FIFO; GpSimd is
    per-Q7 FIFO with fixed partition ownership), so a wait on the instruction's
    own engine proc semaphore is redundant — and walrus can only encode one
    wait on most instruction structs."""
    import re

    from concourse import mybir

    eng_prefix = {
        mybir.EngineType.PE: "PE",
        mybir.EngineType.Activation: "Activation",
        mybir.EngineType.Pool: "Pool",
        mybir.EngineType.DVE: "DVE",
        mybir.EngineType.SP: "SP",
    }
    for f in nc.m.functions:
        for blk in f.blocks:
            new_insts = []
            for inst in blk.instructions:
                si = inst.sync_info
                if (
                    si is None
                    or len(si.on_wait) <= 1
                    or type(inst).__name__ == "InstDrain"
                ):
                    new_insts.append(inst)
                    continue
                keep = list(si.on_wait)
                name = eng_prefix.get(inst.engine)
                if name is not None:
                    pat = re.compile(rf"^{name}_\d+$")
                    keep = [w for w in keep if not pat.match(w.ant_name)]
                # Walrus encodes at most one wait on most instruction structs:
                # move extras onto same-engine NOPs issued just before (the
                # engine executes its queue in order, so waiting on the NOPs
                # first is equivalent).
                extras, keep = keep[1:], keep[:1]
                for w in extras:
                    nop = mybir.InstNoOp(
                        name=nc.get_next_instruction_name(),
                        ins=[],
                        outs=[],
                        engine=inst.engine,
                        sync_info=mybir.SyncInfo(on_wait=[w], on_update=[]),
                        bass_nofuse=True,
                    )
                    new_insts.append(nop)
                inst.sync_info = mybir.SyncInfo(
                    on_wait=keep, on_update=list(si.on_update)
                )
                new_insts.append(inst)
            blk.instructions = new_insts


def _build_nc():
    from concourse import bass, tile, mybir

    F32 = mybir.dt.float32
    BF16 = mybir.dt.bfloat16
    AF = mybir.ActivationFunctionType
    SplitDrainTileContext = make_split_drain_tc()

    nc = bass.Bass()
    zT_d = nc.declare_dram_parameter("zT", [4, 128, BANDC], BF16, isOutput=False)
    aug_d = nc.declare_dram_parameter("aug", [1, BANDC + 128], F32, isOutput=False)
    sqb_d = nc.declare_dram_parameter("sqb", [128, TI], F32, isOutput=False)
    out_d = nc.declare_dram_parameter("out", [128, TI], F32, isOutput=True)

    with SplitDrainTileContext(nc) as tc:
        with (
            tc.tile_pool(name="zpool", bufs=1) as zpool,
            tc.tile_pool(name="small", bufs=1) as small,
            tc.tile_pool(name="lbuf", bufs=2) as lpool,
            tc.tile_pool(name="dbuf", bufs=2) as dpool,
            tc.tile_pool(name="ebuf", bufs=1) as epool,
            tc.tile_pool(name="psum", bufs=2, space="PSUM") as pspool,
        ):
            # Each chunk in two half-tiles (per-tile DMA dependency granularity:
            # one DMA per tile keeps every matmul at <=1 DMA wait).
            HALF = 2560  # units 0..4 in the A half, 5..9 in the B half
            zTa = [
                zpool.tile([128, HALF], BF16, name=f"zTa{ch}", tag=f"zTa{ch}")
                for ch in range(4)
            ]
            zTb = [
                zpool.tile([128, BANDC - HALF], BF16, name=f"zTb{ch}", tag=f"zTb{ch}")
                for ch in range(4)
            ]
            aug = small.tile([1, BANDC + 128], F32)
            sqb = small.tile([128, TI], F32)
            acc = small.tile([128, TI], F32)
            wscr = small.tile([128, TI], F32)

            nc.sync.dma_start(aug[:], aug_d[:])
            nc.sync.dma_start(sqb[:], sqb_d[:])
            for ch in range(4):
                nc.sync.dma_start(zTa[ch][:], zT_d[ch, :, 0:HALF])
            for ch in range(4):
                nc.sync.dma_start(zTb[ch][:], zT_d[ch, :, HALF:BANDC])

            def rhs_slice(ch, u):
                if (u + 1) * 512 <= HALF:
                    return zTa[ch][:, u * 512 : (u + 1) * 512]
                return zTb[ch][:, u * 512 - HALF : (u + 1) * 512 - HALF]

            # Warmup: absorbs the sqb DMA wait into ACT and triggers the
            # natural_log_exp table load immediately.
            nc.scalar.activation(wscr[:], sqb[:], AF.Ln)

            GROUPS = [(0, 4), (4, 4), (8, 1)]  # (start unit, n units) per i-tile
            for t in range(TI):
                u0 = t // 4
                C = 128 * (t % 4)
                lb = lpool.tile([128, NU * 512], F32, tag="lb")
                db = dpool.tile([128, NU * 512], F32, tag="db")

                for gs, gn in GROUPS:
                    ps = pspool.tile([128, 2048], F32, tag="ps")
                    # K=128 chunks, lhsT reused across the group's units
                    for ch in range(4):
                        lhsT = zTa[ch][:, t * 128 : (t + 1) * 128]
                        for j in range(gn):
                            u = u0 + gs + j
                            nc.tensor.matmul(
                                ps[:, j * 512 : (j + 1) * 512],
                                lhsT,
                                rhs_slice(ch, u),
                                start=(ch == 0),
                                stop=False,
                                skip_group_check=True,
                            )
                    for j in range(gn):
                        u = u0 + gs + j
                        nc.tensor.matmul(
                            ps[:, j * 512 : (j + 1) * 512],
                            aug[0:1, BANDC : BANDC + 128],
                            aug[0:1, u * 512 : (u + 1) * 512],
                            start=False,
                            stop=True,
                            skip_group_check=True,
                        )
                    # pass1: L = ln(-2*s + sq_i + EPS)
                    nc.scalar.activation(
                        lb[:, gs * 512 : (gs + gn) * 512],
                        ps[:, 0 : gn * 512],
                        AF.Ln,
                        bias=sqb[:, t : t + 1],
                        scale=-2.0,
                    )

                # Band-edge masks on L (gpsimd, engine otherwise idle):
                # lower edge (unit 0): keep j - i > 0  <=>  f - p - C > 0
                nc.gpsimd.affine_select(
                    lb[:, 0:512],
                    lb[:, 0:512],
                    pattern=[[1, 512]],
                    channel_multiplier=-1,
                    base=-C,
                    compare_op=mybir.AluOpType.is_gt,
                    fill=MASK_FILL,
                )
                # upper edge (unit 8): keep j - i < 4096  <=>  p + C - f > 0
                # (is_lt is unimplemented in walrus codegen; negate for is_gt)
                nc.gpsimd.affine_select(
                    lb[:, 8 * 512 : 9 * 512],
                    lb[:, 8 * 512 : 9 * 512],
                    pattern=[[-1, 512]],
                    channel_multiplier=1,
                    base=C,
                    compare_op=mybir.AluOpType.is_gt,
                    fill=MASK_FILL,
                )

                # pass2: d = exp(0.5 * L) = sqrt(d2)
                nc.scalar.activation(db[:], lb[:], AF.Exp, scale=0.5)

                # pass3: e = exp(-d), per-partition sums into acc[:, t]
                eb = epool.tile([128, NU * 512], BF16, tag="eb")
                nc.scalar.activation(
                    eb[:], db[:], AF.Exp, scale=-1.0, accum_out=acc[:, t : t + 1]
                )

            nc.sync.dma_start(out_d[:], acc[:])

    strip_pe_self_waits(nc)
    return nc


def _get_nc():
    if "nc" not in _CACHE:
        _CACHE["nc"] = _build_nc()
    return _CACHE["nc"]


def _make_in_maps(z: np.ndarray):
    import ml_dtypes

    zd = z.astype(np.float64)
    sq_full = (zd * zd).sum(axis=1)  # [N] fp64
    in_maps = []
    for c in range(NCORES):
        shift = c * ROWS
        rot = np.roll(z, -shift, axis=0)
        sq = np.roll(sq_full, -shift)
        band = rot[:BANDC]  # [BANDC, D]
        zT = (
            np.ascontiguousarray(band.T)
            .reshape(4, 128, BANDC)
            .astype(ml_dtypes.bfloat16)
        )
        augv = np.ones((1, BANDC + 128), np.float32)
        augv[0, :BANDC] = (-0.5 * sq[:BANDC]).astype(np.float32)
        sqb = (sq[:ROWS].reshape(TI, 128).T + EPS).astype(np.float32)
        in_maps.append({"zT": zT, "aug": augv, "sqb": sqb})
    return in_maps


def _run(z: np.ndarray, trace: bool = False):
    from concourse.bass_utils import run_bass_kernel_spmd

    nc = _get_nc()
    in_maps = _make_in_maps(z)
    res = run_bass_kernel_spmd(nc, in_maps, list(range(NCORES)), trace=trace)
    return res


def _postprocess(z: np.ndarray, results) -> np.float32:
    zd = z.astype(np.float64)
    total = 0.0
    for c in range(NCORES):
        total += float(results[c]["out"].astype(np.float64).sum())
    # Undo the EPS shift: d' = sqrt(d2+EPS) ~ d + EPS/(2d); dominant terms
    # have d ~ 32, so scale by exp(+EPS/64).
    total *= float(np.exp(EPS / 64.0))
    # Pairs at separation exactly 4096 (excluded on device), in fp64.
    diff = zd[: N // 2] - zd[N // 2 :]
    dsep = np.sqrt((diff * diff).sum(axis=1))
    total += float(np.exp(-dsep / TAU).sum())
    cnt = N * (N - 1) // 2
    return np.float32(total / cnt)


def kernel(z: np.ndarray) -> np.ndarray:
    z = np.ascontiguousarray(np.asarray(z, dtype=np.float32))
    assert z.shape == (N, D), z.shape
    res = _run(z, trace=False)
    return np.array(_postprocess(z, res.results), dtype=np.float32)


if __name__ == "__main__":
    rng = np.random.default_rng(0)
    z = rng.standard_normal((N, D)).astype(np.float32)
    print(kernel(z))


# revision 15
# speedup vs baseline: 1.7230x; 1.7230x over previous
"""DispersionLoss Trainium2 kernel.

Computes mean over i<j pairs of exp(-||z_i - z_j||) for z [8192, 512] fp32,
distributed over 8 NeuronCores.

Strategy (identical SPMD program on all 8 cores; per-core behavior comes only
from input data):
  - Host rotates z by c*1024 rows for core c, so each core's 1024 rows sit at
    rotated positions 0..1023 and its circulant band of columns is the static
    range 0..5119. Each unordered pair {a,b} with forward separation
    s in [1, 4095] is computed exactly once somewhere in the fleet; the 4096
    pairs at separation exactly 4096 are added on the host in fp64.
  - Per core: 8 i-tiles x 9 j-units of [128, 512]. TensorE computes
    s = dot(z_i, z_j) - (sq_i + sq_j)/2 via 4 K=128 matmuls from a resident
    transposed band plus one K=1 augmented matmul (ones x -sq_j/2).
  - ScalarE (single ACT table set natural_log_exp_and_others, no reloads):
      pass1: L = Ln(-2*s + (sq_i + eps))   [= ln(d2)]
      pass2: d = Exp(0.5*L)                [= sqrt(d2)]
      pass3: e = Exp(-d), accum_out -> per-partition sums
  - GpSimd affine_select masks the band edges (j <= i and j - i >= 4096) by
    overwriting L with +170 so exp(-exp(85)) underflows to exactly 0.
  - Host: fp64 sum of the 8 cores' [128, 8] partial sums, undo the eps bias,
    add the separation-4096 pairs, divide by n(n-1)/2.
"""

import sys

sys.path.insert(0, "/opt/trn_rl_repo")

import numpy as np

N = 8192
D = 512
NCORES = 8
ROWS = N // NCORES          # 1024 rows per core
TI = ROWS // 128            # 8 i-tiles per core
NU = 9                      # j-units of 512 per i-tile
BANDC = (NU + 1) * 512      # 5120 resident band columns
EPS = 0.01                  # diagonal-safety shift added to d2 via the bias
SQC = 1024.0                # centering constant for the bf16 aug row
MASK_FILL = 170.0           # L fill for masked elements: exp(-exp(85)) == 0
TAU = 1.0

_CACHE = {}


def make_split_drain_tc():
    """TileContext subclass whose kernel-tail drain splits its sem waits into
    individual 1-wait NOPs (walrus rejects many waits on one instruction)."""
    from concourse import tile, mybir
    from concourse.vector_clock import ScopedClock

    class SplitDrainTileContext(tile.TileContext):
        def _drain_and_barrier(self, tick_clock, wait_clock):
            drain_inst = self.nc.sync.drain()
            wait_clock.add_sem_waits(
                drain_inst.ins, ScopedClock({None: tick_clock.global_clock})
            )
            si = drain_inst.ins.sync_info
            if si is not None and len(si.on_wait) > 1:
                waits = list(si.on_wait)
                drain_inst.ins.sync_info = mybir.SyncInfo(
                    on_wait=[], on_update=list(si.on_update)
                )
                for w in waits:
                    nop = self.nc.sync.nop(nofuse=True)
                    nop.ins.sync_info = mybir.SyncInfo(on_wait=[w], on_update=[])
            self.nc.all_engine_barrier()
            assert self.sems is not None
            popped = self.nc._tile_sem_poison_stack.pop()
            assert popped is self._sem_poison
            self.nc.clear_and_free_semaphores(list(self.sems.allocated().values()))
            self.nc.all_engine_barrier()

    return SplitDrainTileContext


def strip_pe_self_waits(nc):
    """Drop same-engine self-waits from compute instructions that carry more
    than one wait. Each engine executes and retires its own instruction stream
    in order (PE matmuls are pc-monotone; ACT/DVE are strict FIFO; GpSimd is
    per-Q7 FIFO with fixed partition ownership), so a wait on the instruction's
    own engine proc semaphore is redundant — and walrus can only encode one
    wait on most instruction structs."""
    import re

    from concourse import mybir

    eng_prefix = {
        mybir.EngineType.PE: "PE",
        mybir.EngineType.Activation: "Activation",
        mybir.EngineType.Pool: "Pool",
        mybir.EngineType.DVE: "DVE",
        mybir.EngineType.SP: "SP",
    }
    for f in nc.m.functions:
        for blk in f.blocks:
            new_insts = []
            for inst in blk.instructions:
                si = inst.sync_info
                if (
                    si is None
                    or len(si.on_wait) <= 1
                    or type(inst).__name__ == "InstDrain"
                ):
                    new_insts.append(inst)
                    continue
                keep = list(si.on_wait)
                name = eng_prefix.get(inst.engine)
                if name is not None:
                    pat = re.compile(rf"^{name}_\d+$")
                    keep = [w for w in keep if not pat.match(w.ant_name)]
                # Walrus encodes at most one wait on most instruction structs:
                # move extras onto same-engine NOPs issued just before (the
                # engine executes its queue in order, so waiting on the NOPs
                # first is equivalent).
                extras, keep = keep[1:], keep[:1]
                for w in extras:
                    nop = mybir.InstNoOp(
                        name=nc.get_next_instruction_name(),
                        ins=[],
                        outs=[],
                        engine=inst.engine,
                        sync_info=mybir.SyncInfo(on_wait=[w], on_update=[]),
                        bass_nofuse=True,
                    )
                    new_insts.append(nop)
                inst.sync_info = mybir.SyncInfo(
                    on_wait=keep, on_update=list(si.on_update)
                )
                new_insts.append(inst)
            blk.instructions = new_insts


def _build_nc():
    from concourse import bass, tile, mybir

    F32 = mybir.dt.float32
    BF16 = mybir.dt.bfloat16
    AF = mybir.ActivationFunctionType
    SplitDrainTileContext = make_split_drain_tc()

    nc = bass.Bass()
    zT_d = nc.declare_dram_parameter("zT", [4, 128, BANDC], BF16, isOutput=False)
    aug_d = nc.declare_dram_parameter("aug", [1, BANDC + 128], BF16, isOutput=False)
    sqb_d = nc.declare_dram_parameter("sqb", [128, TI], F32, isOutput=False)
    out_d = nc.declare_dram_parameter("out", [128, TI], F32, isOutput=True)

    with SplitDrainTileContext(nc) as tc:
        with (
            tc.tile_pool(name="zpool", bufs=1) as zpool,
            tc.tile_pool(name="small", bufs=1) as small,
            tc.tile_pool(name="lbuf", bufs=2) as lpool,
            tc.tile_pool(name="dbuf", bufs=2) as dpool,
            tc.tile_pool(name="ebuf", bufs=1) as epool,
            tc.tile_pool(name="psum", bufs=2, space="PSUM") as pspool,
        ):
            # Each chunk in two half-tiles (per-tile DMA dependency granularity:
            # one DMA per tile keeps every matmul at <=1 DMA wait).
            HALF = 2560  # units 0..4 in the A half, 5..9 in the B half
            zTa = [
                zpool.tile([128, HALF], BF16, name=f"zTa{ch}", tag=f"zTa{ch}")
                for ch in range(4)
            ]
            zTb = [
                zpool.tile([128, BANDC - HALF], BF16, name=f"zTb{ch}", tag=f"zTb{ch}")
                for ch in range(4)
            ]
            aug = small.tile([1, BANDC + 128], BF16)
            sqb = small.tile([128, TI], F32)
            acc = small.tile([128, TI], F32)
            wscr = small.tile([128, TI], F32)

            nc.sync.dma_start(aug[:], aug_d[:])
            nc.sync.dma_start(sqb[:], sqb_d[:])
            for ch in range(4):
                nc.sync.dma_start(zTa[ch][:], zT_d[ch, :, 0:HALF])
            for ch in range(4):
                nc.sync.dma_start(zTb[ch][:], zT_d[ch, :, HALF:BANDC])

            def rhs_slice(ch, u):
                if (u + 1) * 512 <= HALF:
                    return zTa[ch][:, u * 512 : (u + 1) * 512]
                return zTb[ch][:, u * 512 - HALF : (u + 1) * 512 - HALF]

            # Warmup: absorbs the sqb DMA wait into ACT and triggers the
            # natural_log_exp table load immediately.
            nc.scalar.activation(wscr[:], sqb[:], AF.Ln)

            GROUPS = [(0, 4), (4, 4), (8, 1)]  # (start unit, n units) per i-tile
            for t in range(TI):
                u0 = t // 4
                C = 128 * (t % 4)
                lb = lpool.tile([128, NU * 512], F32, tag="lb")
                db = dpool.tile([128, NU * 512], F32, tag="db")

                for gs, gn in GROUPS:
                    ps = pspool.tile([128, 2048], F32, tag="ps")
                    # K=128 chunks, lhsT reused across the group's units
                    for ch in range(4):
                        lhsT = zTa[ch][:, t * 128 : (t + 1) * 128]
                        for j in range(gn):
                            u = u0 + gs + j
                            nc.tensor.matmul(
                                ps[:, j * 512 : (j + 1) * 512],
                                lhsT,
                                rhs_slice(ch, u),
                                start=(ch == 0),
                                stop=False,
                                skip_group_check=True,
                            )
                    for j in range(gn):
                        u = u0 + gs + j
                        nc.tensor.matmul(
                            ps[:, j * 512 : (j + 1) * 512],
                            aug[0:1, BANDC : BANDC + 128],
                            aug[0:1, u * 512 : (u + 1) * 512],
                            start=False,
                            stop=True,
                            skip_group_check=True,
                        )
                    # pass1: L = ln(-2*s + sq_i + EPS)
                    nc.scalar.activation(
                        lb[:, gs * 512 : (gs + gn) * 512],
                        ps[:, 0 : gn * 512],
                        AF.Ln,
                        bias=sqb[:, t : t + 1],
                        scale=-2.0,
                    )

                # Band-edge masks on L (gpsimd, engine otherwise idle):
                # lower edge (unit 0): keep j - i > 0  <=>  f - p - C > 0
                nc.gpsimd.affine_select(
                    lb[:, 0:512],
                    lb[:, 0:512],
                    pattern=[[1, 512]],
                    channel_multiplier=-1,
                    base=-C,
                    compare_op=mybir.AluOpType.is_gt,
                    fill=MASK_FILL,
                )
                # upper edge (unit 8): keep j - i < 4096  <=>  p + C - f > 0
                # (is_lt is unimplemented in walrus codegen; negate for is_gt)
                nc.gpsimd.affine_select(
                    lb[:, 8 * 512 : 9 * 512],
                    lb[:, 8 * 512 : 9 * 512],
                    pattern=[[-1, 512]],
                    channel_multiplier=1,
                    base=C,
                    compare_op=mybir.AluOpType.is_gt,
                    fill=MASK_FILL,
                )

                # pass2: d = exp(0.5 * L) = sqrt(d2)
                nc.scalar.activation(db[:], lb[:], AF.Exp, scale=0.5)

                # pass3: e = exp(-d), per-partition sums into acc[:, t]
                eb = epool.tile([128, NU * 512], BF16, tag="eb")
                nc.scalar.activation(
                    eb[:], db[:], AF.Exp, scale=-1.0, accum_out=acc[:, t : t + 1]
                )

            nc.sync.dma_start(out_d[:], acc[:])

    strip_pe_self_waits(nc)
    return nc


def _enable_ldw_opt():
    """Flip walrus's --enable-ldw-opt to true: our matmul groups reuse the
    same stationary operand across consecutive MMs, and deduped LDWEIGHTS
    keeps the PE streaming instead of serializing LDW+MM."""
    if _CACHE.get("ldw_patched"):
        return
    from concourse import bass_utils

    orig = bass_utils.run_command

    def patched(cmd, *a, **kw):
        if isinstance(cmd, list):
            cmd = [
                "--enable-ldw-opt=true" if c == "--enable-ldw-opt=false" else c
                for c in cmd
            ]
        return orig(cmd, *a, **kw)

    bass_utils.run_command = patched
    _CACHE["ldw_patched"] = True


def _get_nc():
    if "nc" not in _CACHE:
        _CACHE["nc"] = _build_nc()
    return _CACHE["nc"]


def _make_in_maps(z: np.ndarray):
    import ml_dtypes

    zd = z.astype(np.float64)
    sq_full = (zd * zd).sum(axis=1)  # [N] fp64
    in_maps = []
    for c in range(NCORES):
        shift = c * ROWS
        rot = np.roll(z, -shift, axis=0)
        sq = np.roll(sq_full, -shift)
        band = rot[:BANDC]  # [BANDC, D]
        zT = (
            np.ascontiguousarray(band.T)
            .reshape(4, 128, BANDC)
            .astype(ml_dtypes.bfloat16)
        )
        # Centered so bf16 rounding of the aug row stays ~1e-1 absolute on d2:
        # aug contributes -2*c_j = sq_j - SQC to d2; SQC restored via the bias.
        augv = np.ones((1, BANDC + 128), np.float64)
        augv[0, :BANDC] = 0.5 * (SQC - sq[:BANDC])
        augv = augv.astype(ml_dtypes.bfloat16)
        sqb = (sq[:ROWS].reshape(TI, 128).T + SQC + EPS).astype(np.float32)
        in_maps.append({"zT": zT, "aug": augv, "sqb": sqb})
    return in_maps


def _run(z: np.ndarray, trace: bool = False):
    from concourse.bass_utils import run_bass_kernel_spmd

    nc = _get_nc()
    in_maps = _make_in_maps(z)
    res = run_bass_kernel_spmd(nc, in_maps, list(range(NCORES)), trace=trace)
    return res


def _postprocess(z: np.ndarray, results) -> np.float32:
    zd = z.astype(np.float64)
    total = 0.0
    for c in range(NCORES):
        total += float(results[c]["out"].astype(np.float64).sum())
    # Undo the EPS shift: d' = sqrt(d2+EPS) ~ d + EPS/(2d); dominant terms
    # have d ~ 32, so scale by exp(+EPS/64).
    total *= float(np.exp(EPS / 64.0))
    # Pairs at separation exactly 4096 (excluded on device), in fp64.
    diff = zd[: N // 2] - zd[N // 2 :]
    dsep = np.sqrt((diff * diff).sum(axis=1))
    total += float(np.exp(-dsep / TAU).sum())
    cnt = N * (N - 1) // 2
    return np.float32(total / cnt)


def kernel(z: np.ndarray) -> np.ndarray:
    z = np.ascontiguousarray(np.asarray(z, dtype=np.float32))
    assert z.shape == (N, D), z.shape
    res = _run(z, trace=False)
    return np.array(_postprocess(z, res.results), dtype=np.float32)


if __name__ == "__main__":
    rng = np.random.default_rng(0)
    z = rng.standard_normal((N, D)).astype(np.float32)
    print(kernel(z))
